# revision 27
# baseline (speedup 1.0000x reference)
"""Trainium2 Bass kernel for Autoformer-style autocorrelation attention.

Math (matches the reference nn.Module):
    top_k = int(log(L)) = 6
    mean_value[b, l] = corr[b].mean(over H, C)                     # [B, L]
    idx = top_k(mean_value.mean(over B))                           # [6]
    w = softmax(mean_value[:, idx], axis=-1)                       # [B, 6]
    out[b, h, c, l] = sum_k w[b, k] * values[b, h, c, (l+idx_k)%L]

Strategy: data-parallel over B (4 batches per core on 8 cores), two
launches with tiny host glue (top-k + softmax) in between.

Launch 1 reduces corr over (H, C) per batch via ones-matmuls on the
tensor engine.  corr goes in as fp16 (quantization ~1e-5 on the means,
far below the 1.1e-4 top-k margin; fp8 flips the top-k — verified on the
actual distribution).  Each batch is one [128, 4096] SBUF tile whose
partition line holds 4 consecutive HBM rows (one contiguous 8 KiB DMA
descriptor per partition; measured ~390 GB/s vs ~341 with 2 KiB descs).
The input DMAs are issued before anything else so the HBM stream starts
as early as the DGE allows; the last batch is split into two
column-halves so its matmuls start one transfer earlier (it alone sets
the tail).  Eight PSUM banks hold the 4x2 per-(batch, half)
accumulators; the DVE and scalar engines copy the halves' row 0 to SBUF
in parallel and one merged DMA writes all sums back.

Launch 2 bakes the 6 indices in as static SBUF column windows and
splits the 6 weighted-shift terms across all four compute engines so no
single engine is far above the DMA roofline:
  - PE: 4 shifts as diag(w_bk) @ window matmuls accumulating in PSUM
    (diags are built ON DEVICE: gpsimd affine_select makes a [128,128]
    fp16 identity, DVE tensor_scalar scales it by w_bk from a 12 KiB
    broadcast weight input — nothing big crosses HBM for them).
  - DVE: scalar_tensor_tensor fuses shift #5 with the PSUM merge
    (acc16 = w4*shifted + psum), reading the [128, 1024] psum tile
    across both banks so only the shift's wrap splits it, writing fp16.
  - scalar engine (ACT): shift #6 as a scaled copy (t6 = w5*shifted).
  - final add out16 = acc16 + t6 split by columns between DVE
    tensor_tensor (2x fp16 pipe) and gpsimd tensor_tensor (the only
    dense op the Pool engine's ISA accepts — TensorScalarPtr is
    rejected by neuronxcc on Pool, and ACT->PSUM-then-accumulate races
    the matmuls nondeterministically; both verified on hardware).
All 16 value tiles are buffered in SBUF (4 MiB) with every input
trigger issued ahead of the out-triggers on SP, so the input stream
runs at full rate and a waiting out-trigger never delays compute.
The output is written fp16 (halves write traffic; quantization ~5e-4
relative, far under the gate) and upcast to fp32 on the host.  A warmup
burst of junk matmuls defeats the HAM cold clock during the DMA ramp
(fewer than ~8 delays the clock boost measurably).
Per-batch weights enter through an input tensor so one compiled NEFF is
SPMD across all 8 cores.
"""

import math

import numpy as np

_B, _H, _C, _L = 32, 8, 64, 1024
_NCORES = 8
_BLOC = _B // _NCORES  # batches per core
_R = _H * _C           # rows per batch
_PART = 128
_TPB = _R // _PART     # SBUF tiles per batch
_TOPK = int(math.log(_L))  # 6
_NPE = 4               # shift terms handled by the tensor engine
_HALF = 512            # PSUM bank width in fp32


def _build_phase1():
    import concourse.bacc as bacc
    import concourse.mybir as mybir
    import concourse.tile as tile

    f32 = mybir.dt.float32
    f16 = mybir.dt.float16
    nc = bacc.Bacc("TRN2", target_bir_lowering=False, debug=False,
                   enable_partition_id=False)
    corr_d = nc.dram_tensor("corr_sh", [_BLOC, _R, _L], f16, kind="ExternalInput").ap()
    sums_d = nc.dram_tensor("sums", [1, _BLOC * _L], f32, kind="ExternalOutput").ap()

    with tile.TileContext(nc) as tc:
        with (
            tc.tile_pool(name="io", bufs=4) as io_pool,
            tc.tile_pool(name="const", bufs=1) as const_pool,
            tc.tile_pool(name="acc", bufs=1) as acc_pool,
            tc.tile_pool(name="ps", bufs=1, space="PSUM") as ps_pool,
        ):
            # input DMAs first: partition p <- 4 consecutive HBM rows (one
            # contiguous 8 KiB descriptor per partition line, ~390 GB/s).
            # The LAST batch is split into two column-halves (4 KiB descs)
            # so its first matmuls start one transfer earlier — it alone
            # sets the tail.
            vts = {}
            for b in range(_BLOC - 1):
                vt = io_pool.tile([_PART, _TPB * _L], f16, tag="vt", bufs=3,
                                  name=f"vt{b}")
                nc.sync.dma_start(
                    vt[:], corr_d[b, :, :].rearrange("(p f) l -> p (f l)",
                                                     p=_PART))
                vts[b] = vt
            b3_ap = corr_d[_BLOC - 1, :, :].rearrange("(p f) l -> p (f l)",
                                                      p=_PART)
            b3 = []
            for u in range(2):
                vt = io_pool.tile([_PART, 2 * _L], f16, tag="vtl", bufs=2,
                                  name=f"vtl{u}")
                nc.sync.dma_start(vt[:], b3_ap[:, u * 2 * _L:(u + 1) * 2 * _L])
                b3.append(vt)

            ones = const_pool.tile([_PART, _HALF], f16)
            nc.vector.memset(ones[:], 1.0)
            junk = const_pool.tile([_PART, _HALF], f16)
            outs = acc_pool.tile([1, _BLOC * _L], f32)
            pss = {}
            for b in range(_BLOC):
                for h in range(2):
                    pss[b, h] = ps_pool.tile([_PART, _HALF], f32,
                                             tag=f"ps{b}{h}", name=f"ps{b}{h}")
            # HAM warmup: junk work on EVERY engine so the activity monitor
            # lifts the core clock while the first corr tile streams in
            # (the matmuls share bank (0,0); its start=True resets)
            for _ in range(8):
                nc.tensor.matmul(pss[0, 0][:], ones[:, 0:_PART], ones[:],
                                 start=True, stop=True)
            for _ in range(4):
                nc.vector.tensor_scalar_mul(junk[:], ones[:], 1.0)
            for _ in range(3):
                nc.scalar.copy(junk[:], ones[:])
            for _ in range(2):
                nc.gpsimd.tensor_tensor(junk[:], ones[:], ones[:],
                                        op=mybir.AluOpType.add)
            for b in range(_BLOC):
                for f in range(_TPB):
                    for h in range(2):
                        if b < _BLOC - 1:
                            rhs = vts[b][:, f * _L + h * _HALF:
                                         f * _L + (h + 1) * _HALF]
                        else:
                            rhs = b3[f // 2][:, (f % 2) * _L + h * _HALF:
                                             (f % 2) * _L + (h + 1) * _HALF]
                        nc.tensor.matmul(
                            pss[b, h][:],
                            ones[:, 0:_PART],
                            rhs,
                            start=(f == 0),
                            stop=(f == _TPB - 1),
                        )
                o0 = b * _L
                # one half on DVE, the other on the scalar engine (parallel)
                nc.vector.tensor_scalar_mul(
                    outs[0:1, o0:o0 + _HALF], pss[b, 0][0:1, :], 1.0)
                nc.scalar.copy(outs[0:1, o0 + _HALF:o0 + _L], pss[b, 1][0:1, :])
            # single write-back of all four batches' sums
            nc.scalar.dma_start(sums_d[0:1, :], outs[0:1, :])
    nc.compile()
    return nc


def _wrap_pieces(s, c0, c1):
    """Split out-column range [c0, c1) of a shift-by-s read into
    (out_off, n, src_off) pieces that stay contiguous in the source."""
    pieces = []
    c = c0
    while c < c1:
        src = (c + s) % _L
        n = min(c1 - c, _L - src)
        pieces.append((c, n, src))
        c += n
    return pieces


def _build_phase2(idx):
    import concourse.bacc as bacc
    import concourse.mybir as mybir
    import concourse.tile as tile

    f32 = mybir.dt.float32
    f16 = mybir.dt.float16
    alu = mybir.AluOpType
    act_copy = mybir.ActivationFunctionType.Copy

    # engine assignment: the scalar engine (ACT) gets the index with the
    # fewest wrap pieces, DVE the one whose pieces best align with the
    # PSUM halves, PE the remaining four
    srt = sorted(idx, key=lambda s: (s != 0, min(s % _L, _L - s)))
    s_act = srt[0]
    rest = [s for s in idx if s != s_act]
    s_dve = sorted(rest, key=lambda s: (s % _HALF != 0,
                                        min(s % _HALF, _HALF - s % _HALF)))[0]
    s_pe = [s for s in rest if s != s_dve]
    assert len(s_pe) == _NPE
    k_of = {s: k for k, s in enumerate(idx)}
    # final-add column split between DVE (2x tt) and gpsimd (0.42x tt)
    _DCOL = 384

    nc = bacc.Bacc("TRN2", target_bir_lowering=False, debug=False,
                   enable_partition_id=False)
    vals_d = nc.dram_tensor("vals", [_BLOC, _R, _L], f16, kind="ExternalInput").ap()
    wsb_d = nc.dram_tensor("wsb", [_PART, _BLOC * _TOPK], f32, kind="ExternalInput").ap()
    out_d = nc.dram_tensor("out_sh", [_BLOC, _R, _L], f16, kind="ExternalOutput").ap()

    ntiles = _BLOC * _TPB
    with tile.TileContext(nc) as tc:
        with (
            tc.tile_pool(name="const", bufs=1) as const_pool,
            tc.tile_pool(name="v16", bufs=ntiles) as v16_pool,
            tc.tile_pool(name="t6p", bufs=4) as t6_pool,
            tc.tile_pool(name="acc", bufs=4) as acc_pool,
            tc.tile_pool(name="out", bufs=4) as out_pool,
            tc.tile_pool(name="ps", bufs=3, space="PSUM") as ps_pool,
        ):
            # weights first on the wire (12 KiB, lands well before tile 0)
            w_t = const_pool.tile([_PART, _BLOC * _TOPK], f32)
            nc.sync.dma_start(w_t[:], wsb_d[:])

            # fully-buffered input stream: 16 tiles (4 MiB SBUF), first 8
            # triggers upfront, the rest interleaved ahead of out-triggers
            vts = [v16_pool.tile([_PART, _L], f16, tag="vt16", name=f"vt{j}")
                   for j in range(ntiles)]

            def in_trig(j):
                b, t = divmod(j, _TPB)
                nc.sync.dma_start(
                    vts[j][:], vals_d[b, t * _PART:(t + 1) * _PART, :])

            _PREF = 8
            for j in range(_PREF):
                in_trig(j)

            # HAM warmup: junk work on EVERY engine so the activity monitor
            # lifts the core clock before the first real tile lands
            wones = const_pool.tile([_PART, _HALF], f16)
            nc.vector.memset(wones[:], 1.0)
            wjunk = const_pool.tile([_PART, _HALF], f16)
            wps = ps_pool.tile([_PART, _HALF], f32, tag="wps", name="wps", bufs=1)
            for _ in range(8):
                nc.tensor.matmul(wps[:], wones[:, 0:_PART], wones[:],
                                 start=True, stop=True)
            for _ in range(4):
                nc.vector.tensor_scalar_mul(wjunk[:], wones[:], 1.0)
            for _ in range(3):
                nc.scalar.copy(wjunk[:], wones[:])
            for _ in range(2):
                nc.gpsimd.tensor_tensor(wjunk[:], wones[:], wones[:],
                                        op=alu.add)

            # identity on gpsimd, then per-(b, k) scaled diags on DVE
            eye = const_pool.tile([_PART, _PART], f16)
            nc.gpsimd.memset(eye[:], 1.0)
            nc.gpsimd.affine_select(
                eye[:], eye[:], pattern=[[1, _PART]],
                compare_op=alu.is_equal, fill=0.0,
                base=0, channel_multiplier=-1)
            diags = {}
            for b in range(_BLOC):
                for ki, s in enumerate(s_pe):
                    d = const_pool.tile([_PART, _PART], f16, tag=f"d{b}{ki}")
                    nc.vector.tensor_scalar_mul(
                        d[:], eye[:], w_t[:, b * _TOPK + k_of[s]:b * _TOPK + k_of[s] + 1])
                    diags[b, s] = d

            for b in range(_BLOC):
                w_dve = w_t[:, b * _TOPK + k_of[s_dve]:b * _TOPK + k_of[s_dve] + 1]
                w_act = w_t[:, b * _TOPK + k_of[s_act]:b * _TOPK + k_of[s_act] + 1]
                for t in range(_TPB):
                    j = b * _TPB + t
                    if j + _PREF < ntiles:
                        in_trig(j + _PREF)
                    vt16 = vts[j][:, :]
                    out_ap = out_d[b, t * _PART:(t + 1) * _PART, :]

                    # one [128, L] psum tile spanning two banks; matmul
                    # pieces stay within a bank, the DVE read crosses them
                    ps = ps_pool.tile([_PART, _L], f32, tag="ps",
                                      name="ps", bufs=3)
                    pieces = {0: [], 1: []}
                    for s in s_pe:
                        for h in range(2):
                            for (c, n, src) in _wrap_pieces(s, h * _HALF,
                                                            (h + 1) * _HALF):
                                pieces[h].append((s, c, n, src))
                    for h in range(2):
                        for pi, (s, c, n, src) in enumerate(pieces[h]):
                            nc.tensor.matmul(
                                ps[:, c:c + n], diags[b, s][:],
                                vt16[:, src:src + n],
                                start=(pi == 0), stop=(pi == len(pieces[h]) - 1),
                            )

                    # shift 5 fused with the PSUM merge on DVE (fp16 out)
                    acc16 = acc_pool.tile([_PART, _L], f16, tag="acc16")
                    for (c, n, src) in _wrap_pieces(s_dve, 0, _L):
                        nc.vector.scalar_tensor_tensor(
                            acc16[:, c:c + n],
                            vt16[:, src:src + n],
                            w_dve,
                            ps[:, c:c + n],
                            op0=alu.mult,
                            op1=alu.add,
                        )

                    # shift 6 on the scalar engine: t6 = w_act * roll(v)
                    t6 = t6_pool.tile([_PART, _L], f16, tag="t6")
                    for (c, n, src) in _wrap_pieces(s_act, 0, _L):
                        nc.scalar.activation(
                            t6[:, c:c + n], vt16[:, src:src + n],
                            act_copy, scale=w_act)

                    # final add acc16 + t6, columns split DVE (2x) / gpsimd.
                    # The last two tiles sit in the drain shadow: give DVE
                    # (3x faster at tt) most of the columns and write the
                    # halves as independent tiles so each engine's DMA
                    # fires the moment its half is done.
                    if j >= ntiles - 2:
                        dc = 768
                        otA = out_pool.tile([_PART, dc], f16, tag="otA",
                                            name=f"otA{j}", bufs=2)
                        otB = out_pool.tile([_PART, _L - dc], f16, tag="otB",
                                            name=f"otB{j}", bufs=2)
                        nc.vector.tensor_tensor(
                            otA[:], acc16[:, 0:dc], t6[:, 0:dc], op=alu.add)
                        nc.gpsimd.tensor_tensor(
                            otB[:], acc16[:, dc:_L], t6[:, dc:_L], op=alu.add)
                        r0, r1 = t * _PART, (t + 1) * _PART
                        nc.sync.dma_start(out_d[b, r0:r1, 0:dc], otA[:])
                        nc.sync.dma_start(out_d[b, r0:r1, dc:_L], otB[:])
                    else:
                        ot16 = out_pool.tile([_PART, _L], f16, tag="ot16")
                        nc.vector.tensor_tensor(
                            ot16[:, 0:_DCOL], acc16[:, 0:_DCOL],
                            t6[:, 0:_DCOL], op=alu.add)
                        nc.gpsimd.tensor_tensor(
                            ot16[:, _DCOL:_L], acc16[:, _DCOL:_L],
                            t6[:, _DCOL:_L], op=alu.add)
                        # out-triggers on SP: a waiting trigger only delays
                        # later out-triggers (not ready anyway), not compute
                        nc.sync.dma_start(out_ap, ot16[:])
    nc.compile()
    return nc


def _run_spmd(nc, in_maps, **kwargs):
    from concourse import bass_utils

    return bass_utils.run_bass_kernel_spmd(
        nc, in_maps, core_ids=list(range(_NCORES)), **kwargs
    )


def kernel(values: np.ndarray, corr: np.ndarray, _collect=None) -> np.ndarray:
    assert values.shape == (_B, _H, _C, _L) and corr.shape == (_B, _H, _C, _L)
    corr16 = np.ascontiguousarray(
        np.asarray(corr, dtype=np.float32).reshape(_B, _R, _L), dtype=np.float16
    )
    vals16 = np.ascontiguousarray(
        np.asarray(values, dtype=np.float32).reshape(_B, _R, _L), dtype=np.float16
    )

    # ---- launch 1: per-batch sums of corr over (H, C) ----
    nc1 = _build_phase1()
    in1 = [
        {"corr_sh": corr16[c * _BLOC:(c + 1) * _BLOC]}
        for c in range(_NCORES)
    ]
    res1 = _run_spmd(nc1, in1, **(_collect.kwargs(1) if _collect else {}))
    if _collect is not None:
        _collect.add(1, nc1, res1)
    sums = np.concatenate(
        [r["sums"].reshape(_BLOC, _L) for r in res1.results], axis=0
    )  # [B, L]

    # ---- host glue: top-k indices + softmax weights (tiny) ----
    mean_value = sums / np.float32(_R)                       # [B, L]
    g = mean_value.astype(np.float64).mean(axis=0)           # [L]
    idx = np.argsort(-g, kind="stable")[:_TOPK].astype(np.int64)
    wsel = mean_value[:, idx].astype(np.float32)             # [B, 6]
    e = np.exp(wsel - wsel.max(axis=-1, keepdims=True))
    w = (e / e.sum(axis=-1, keepdims=True)).astype(np.float32)

    # ---- launch 2: weighted shifted-gather combine ----
    nc2 = _build_phase2([int(i) for i in idx])
    in2 = []
    for c in range(_NCORES):
        wloc = w[c * _BLOC:(c + 1) * _BLOC]                  # [BLOC, 6]
        wsb = np.ascontiguousarray(
            np.broadcast_to(wloc.reshape(-1)[None, :], (_PART, _BLOC * _TOPK)),
            dtype=np.float32,
        )
        in2.append({
            "vals": vals16[c * _BLOC:(c + 1) * _BLOC],
            "wsb": wsb,
        })
    res2 = _run_spmd(nc2, in2, **(_collect.kwargs(2) if _collect else {}))
    if _collect is not None:
        _collect.add(2, nc2, res2)
    out = np.concatenate([r["out_sh"] for r in res2.results], axis=0)
    return out.reshape(_B, _H, _C, _L).astype(np.float32)
